# revision 6
# baseline (speedup 1.0000x reference)
"""Trainium2 Bass kernel for nn_Bspline_segment_calc.

Math: the reference builds a FIXED uniform extended grid (the `grid` input is
unused): knots g_i = -1.6 + 0.2*i, i = 0..16.  With u = 5*x + 8 (x in [0,1) =>
u in [8,13)), every output row is a shift of the cardinal cubic B-spline
kernel:  out[a, r, n] = M4(u - r),  r = 0..12.  Rows 0..4 are identically zero
(assembled host-side; never touched by the device).

Using the symmetry M4(s) = M4(4-s), with a = |u - (r+2)| (folded distance from
the support center) and z = relu(c*(2-a)) where c^3 = 1/6:

    out = z^3 - 4 * relu(z - c)^3

Edge rows 5 and 12 intersect only one polynomial piece over u in [8,13):
    out_5  = relu(c*(9-u))^3 = cube(relu(c - 5c*x))
    out_12 = relu(c*(u-12))^3 = cube(relu(5c*x - 4c))

Engine split (per chunk) so no engine exceeds the DMA stream time:
  - ScalarE: z for rows 6..9 (abs act + relu act each)        -> 8 acts
  - Pool   : z for rows 10,11 (ts/stt 4-op sequence each)     -> 8 ops
  - DVE    : edge_cube rows 5,12 + cube_diff over pair tiles  -> 5 ops
The cube_diff is evaluated once per ROW-PAIR over a stacked [128, 2*sub]
z tile, which also makes the output stores contiguous 2-row blocks.

Layout: each core's [5, 62500] shard is flattened and padded to 128x2442
(pad value 10.0 maps to basis == 0).  The free dim is processed in graded
chunks (small first chunk => compute and stores start early).  Output DRAM
is organized per chunk as [128, 8, sub] blocks (partition-major, row slots
5..12), so each pair store writes 2*sub*4 contiguous bytes per partition
(bigger DMA lines amortize per-packet overhead).  Host reassembles.

Sharding: x is split along N across the 8 cores; each core computes its 8
nonzero basis rows; host assembles the full [5, 13, 500000] output.
"""

import numpy as np

import concourse.bass as bass
import concourse.bacc as bacc
import concourse.tile as tile
from concourse import mybir
from concourse.alu_op_type import AluOpType
from concourse.bass_utils import run_bass_kernel_spmd
import concourse.dve_ops as dve_ops_mod
from concourse.dve_spec import (
    Spec, Src0, C0, C1, C2, Zero, One, relu, sq, maxx, lower, _has_src1,
)
from concourse.dve_uop import DveOpSpec

N_CORES = 8
N_ROWS = 5          # x rows
N_BASIS = 13        # output basis rows (rows 0..4 are zero)
R_LO = 5            # first nonzero basis row
N_NZ = N_BASIS - R_LO                # 8 nonzero rows
N_FULL = 500000
N_SHARD = N_FULL // N_CORES          # 62500
N_ELEM = N_ROWS * N_SHARD            # 312500 elements per core
P = 128                              # SBUF partitions (all 16 DMA engines)
FD = -(-N_ELEM // P)                 # 2442 elements per partition
N_PAD = P * FD                       # 312576
X_PAD_VAL = np.float32(10.0)         # maps to u far outside every support
C1V = float(np.float64(6.0) ** (-1.0 / 3.0))   # c with c^3 = 1/6
CHUNKS = (256, 1093, 1093)           # graded: small first chunk
S_ROWS = (6, 7, 8, 9)                # z computed on ScalarE
POOL_ROWS = (10, 11)                 # z computed on Pool (gpsimd)
SKIP_INIT_BARRIER = True
ENABLE_ASSERTS = True

assert sum(CHUNKS) == FD


def _chunks():
    bounds = [0]
    for c in CHUNKS:
        bounds.append(bounds[-1] + c)
    return list(zip(bounds[:-1], bounds[1:]))


def _register_dve_op(name, spec):
    for op in dve_ops_mod.OPS:
        if op.name == name:
            return op
    opcode = dve_ops_mod._CUSTOM_DVE_ROW_BASE + len(dve_ops_mod.OPS)
    assert opcode < 0x20, "custom DVE row overflow"
    shas = {}
    for ver in ("v3", "v4"):
        uops = lower(spec, ver=ver)
        shas[ver] = DveOpSpec(
            name=name, opcode=opcode, uops=uops, rd1_en=_has_src1(spec)
        ).sha(ver)
    op = dve_ops_mod.DveOp(name, spec, subdim=False, uops_sha=shas)
    dve_ops_mod.OPS.append(op)
    dve_ops_mod._SUB_OPCODE_FOR_NAME[name] = opcode
    dve_ops_mod.CUSTOM_DVE_SPECS[name] = spec
    return op


def _get_cube_diff_op():
    # out = in0^3 - imm2 * relu(in0 - s0)^3        (8 ALU stages)
    r = relu(Src0 - C0)
    body = sq(Src0) * Src0 - sq(r) * r * C2
    spec = Spec(
        body=body,
        reference=lambda in0, in1, s0, s1, imm2: (
            in0.astype(np.float32) ** 3
            - np.maximum(in0 - s0, np.float32(0.0)).astype(np.float32) ** 3 * imm2
        ).astype(np.float32),
    )
    return _register_dve_op("BSPLINE_CUBE_DIFF_ANT", spec)


def _get_edge_cube_op():
    # out = relu(in0*s0 + s1)^3                    (5 ALU stages)
    r = relu(Src0 * C0 + C1)
    spec = Spec(
        body=sq(r) * r,
        reference=lambda in0, in1, s0, s1, imm2: (
            np.maximum(in0 * s0 + s1, np.float32(0.0)).astype(np.float32) ** 3
        ).astype(np.float32),
    )
    return _register_dve_op("BSPLINE_EDGE_CUBE_ANT", spec)


def _register_const(nc, value):
    """Make `value` usable as an activation bias (const_aps lookup).
    Must be called inside the TileContext: the memset is tracked by Tile."""
    f32 = mybir.dt.float32
    key = (f32, float(value))
    if key in nc.const_aps.aps:
        return
    t = nc.alloc_sbuf_tensor(f"const-f32-{float(value)}", [128, 1], f32)
    nc.vector.memset(t.ap(), float(value))
    nc.const_aps.aps[key] = t.ap()


def _build_bass():
    cube_diff_op = _get_cube_diff_op()
    edge_cube_op = _get_edge_cube_op()
    f32 = mybir.dt.float32
    # Skip Bass.__init__'s trailing all-engine barrier (only guards its
    # 0.0/1.0 const memsets; the earlier _nrt_pseudo_barrier already orders
    # the semaphore clears).  The only in-kernel reader of those consts is
    # the throwaway table-warm activation below.  Saves ~2us of preamble.
    if SKIP_INIT_BARRIER:
        _orig_barrier = bass.Bass.all_engine_barrier
        bass.Bass.all_engine_barrier = lambda self: None
        try:
            nc = bacc.Bacc(
                "TRN2", target_bir_lowering=False, debug=False,
                num_devices=N_CORES, enable_asserts=ENABLE_ASSERTS,
            )
        finally:
            bass.Bass.all_engine_barrier = _orig_barrier
    else:
        nc = bacc.Bacc(
            "TRN2", target_bir_lowering=False, debug=False,
            num_devices=N_CORES, enable_asserts=ENABLE_ASSERTS,
        )
    x_dram = nc.dram_tensor("x", [N_PAD], f32, kind="ExternalInput")
    out_dram = nc.dram_tensor("out", [N_NZ * N_PAD], f32, kind="ExternalOutput")
    xv = x_dram.ap().rearrange("(p f) -> p f", p=P)

    with tile.TileContext(nc) as tc:
        with (
            tc.tile_pool(name="const", bufs=1) as cpool,
            tc.tile_pool(name="work", bufs=2) as wpool,
        ):
            x_tile = cpool.tile([P, FD], f32, tag="x")
            for (lo, hi) in _chunks():
                nc.sync.dma_start(out=x_tile[:, lo:hi], in_=xv[:, lo:hi])

            warm = cpool.tile([P, 1], f32, tag="warm")
            nc.scalar.activation(
                warm[:], nc.const_aps.aps[(f32, 0.0)][:P, :],
                mybir.ActivationFunctionType.Abs, bias=0.0, scale=1.0,
            )
            for r in S_ROWS:
                _register_const(nc, float(6 - r))
            _register_const(nc, 2.0 * C1V)

            for (lo, hi) in _chunks():
                sub = hi - lo
                xs = x_tile[:, lo:hi]
                base = N_NZ * P * lo
                blk = out_dram.ap()[base: base + N_NZ * P * sub].rearrange(
                    "(p f) -> p f", p=P
                )  # [P, 8*sub], per-partition layout = row slots 5..12

                # --- DVE: edge rows straight from x ---
                o5 = wpool.tile([P, sub], f32, tag="o5", bufs=2)
                nc.vector._custom_dve(
                    edge_cube_op, out=o5[:], in0=xs,
                    s0=-5.0 * C1V, s1=C1V,
                )
                nc.sync.dma_start(out=blk[:, 0:sub], in_=o5[:])
                o12 = wpool.tile([P, sub], f32, tag="o12", bufs=2)
                nc.vector._custom_dve(
                    edge_cube_op, out=o12[:], in0=xs,
                    s0=5.0 * C1V, s1=-4.0 * C1V,
                )
                nc.sync.dma_start(out=blk[:, 7 * sub: 8 * sub], in_=o12[:])

                # --- ScalarE: z for rows 6..9 into stacked pair tiles ---
                z67 = wpool.tile([P, 2 * sub], f32, tag="z67", bufs=2)
                z89 = wpool.tile([P, 2 * sub], f32, tag="z89", bufs=2)
                for k, r in enumerate(S_ROWS):
                    zt = z67 if k < 2 else z89
                    off = (k % 2) * sub
                    a_t = wpool.tile([P, sub], f32, tag="a", bufs=3)
                    nc.scalar.activation(
                        a_t[:], xs, mybir.ActivationFunctionType.Abs,
                        bias=float(6 - r), scale=5.0,
                    )
                    nc.scalar.activation(
                        zt[:, off:off + sub], a_t[:],
                        mybir.ActivationFunctionType.Relu,
                        bias=2.0 * C1V, scale=-C1V,
                    )
                    if k == 1:
                        o67 = wpool.tile([P, 2 * sub], f32, tag="o67", bufs=2)
                        nc.vector._custom_dve(
                            cube_diff_op, out=o67[:], in0=z67[:],
                            s0=C1V, imm2=4.0,
                        )
                        nc.sync.dma_start(
                            out=blk[:, sub: 3 * sub], in_=o67[:]
                        )

                # --- Pool: z for rows 10,11 into a stacked pair tile ---
                z1011 = wpool.tile([P, 2 * sub], f32, tag="z1011", bufs=2)
                for k, r in enumerate(POOL_ROWS):
                    off = k * sub
                    # z = relu(min(A, B)), A = c(2-w), B = c(2+w), w = 5x+(6-r)
                    # min(A,B) = [B - 2bc] + min(-10c*x, 2bc)
                    # (Pool tensor_tensor has no fp32 min; tensor_scalar does)
                    b = float(6 - r)
                    t1 = wpool.tile([P, sub], f32, tag="p1", bufs=2)
                    nc.gpsimd.tensor_scalar(
                        t1[:], xs, 5.0 * C1V, (2.0 - b) * C1V,
                        AluOpType.mult, AluOpType.add,
                    )
                    t2 = wpool.tile([P, sub], f32, tag="p2", bufs=2)
                    nc.gpsimd.tensor_scalar(
                        t2[:], xs, -10.0 * C1V, 2.0 * b * C1V,
                        AluOpType.mult, AluOpType.min,
                    )
                    m_p = wpool.tile([P, sub], f32, tag="pm", bufs=2)
                    nc.gpsimd.tensor_tensor(
                        m_p[:], t1[:], t2[:], AluOpType.add,
                    )
                    nc.gpsimd.tensor_scalar(
                        z1011[:, off:off + sub], m_p[:], 0.0, None,
                        AluOpType.max,
                    )

                o1011 = wpool.tile([P, 2 * sub], f32, tag="o1011", bufs=2)
                nc.vector._custom_dve(
                    cube_diff_op, out=o1011[:], in0=z1011[:],
                    s0=C1V, imm2=4.0,
                )
                nc.sync.dma_start(out=blk[:, 5 * sub: 7 * sub], in_=o1011[:])

                o89 = wpool.tile([P, 2 * sub], f32, tag="o89", bufs=2)
                nc.vector._custom_dve(
                    cube_diff_op, out=o89[:], in0=z89[:],
                    s0=C1V, imm2=4.0,
                )
                nc.sync.dma_start(out=blk[:, 3 * sub: 5 * sub], in_=o89[:])
    nc.compile()
    return nc


_NC_CACHE = None


def _get_nc():
    global _NC_CACHE
    if _NC_CACHE is None:
        _NC_CACHE = _build_bass()
    return _NC_CACHE


def kernel(x, grid=None, k=None, **_ignored):
    x = np.asarray(x, dtype=np.float32)
    assert x.shape == (N_ROWS, N_FULL), x.shape
    nc = _get_nc()
    in_maps = []
    for i in range(N_CORES):
        sh = np.full(N_PAD, X_PAD_VAL, dtype=np.float32)
        sh[:N_ELEM] = np.ascontiguousarray(
            x[:, i * N_SHARD: (i + 1) * N_SHARD]
        ).reshape(-1)
        in_maps.append({"x": sh})
    res = run_bass_kernel_spmd(nc, in_maps, list(range(N_CORES))).results
    full = np.zeros((N_ROWS, N_BASIS, N_FULL), dtype=np.float32)
    for i in range(N_CORES):
        o = np.asarray(res[i]["out"]).reshape(-1)  # [8 * N_PAD] chunk blocks
        r8 = np.empty((N_NZ, P, FD), dtype=np.float32)
        for (lo, hi) in _chunks():
            sub = hi - lo
            blk = o[N_NZ * P * lo: N_NZ * P * hi].reshape(P, N_NZ, sub)
            r8[:, :, lo:hi] = blk.transpose(1, 0, 2)
        rows = r8.reshape(N_NZ, N_PAD)[:, :N_ELEM]
        full[:, R_LO:, i * N_SHARD: (i + 1) * N_SHARD] = rows.reshape(
            N_NZ, N_ROWS, N_SHARD
        ).transpose(1, 0, 2)
    return full


# revision 10
# speedup vs baseline: 4.1865x; 4.1865x over previous
"""Trainium2 Bass kernel for nn_Bspline_segment_calc.

Math: the reference builds a FIXED uniform extended grid (the `grid` input is
unused): knots g_i = -1.6 + 0.2*i, i = 0..16.  With u = 5*x + 8 (x in [0,1) =>
u in [8,13)), every output row is a shift of the cardinal cubic B-spline
kernel:  out[a, r, n] = M4(u - r),  r = 0..12.  Rows 0..4 are identically zero
(assembled host-side; never touched by the device).

Using the symmetry M4(s) = M4(4-s), with a = |u - (r+2)| (folded distance from
the support center) and z = relu(c*(2-a)) where c^3 = 1/6:

    out = z^3 - 4 * relu(z - c)^3

Edge rows 5 and 12 intersect only one polynomial piece over u in [8,13):
    out_5  = relu(c*(9-u))^3 = cube(relu(c - 5c*x))
    out_12 = relu(c*(u-12))^3 = cube(relu(5c*x - 4c))

Engine split (per chunk) so no engine exceeds the DMA stream time:
  - ScalarE: z for rows 6..9 (abs act + relu act each)        -> 8 acts
  - Pool   : z for rows 10,11 (ts/stt 4-op sequence each)     -> 8 ops
  - DVE    : edge_cube rows 5,12 + cube_diff over pair tiles  -> 5 ops
The cube_diff is evaluated once per ROW-PAIR over a stacked [128, 2*sub]
z tile, which also makes the output stores contiguous 2-row blocks.

Layout: each core's [5, 62500] shard is flattened and padded to 128x2442
(pad value 10.0 maps to basis == 0).  The free dim is processed in graded
chunks (small first chunk => compute and stores start early).  Output DRAM
is organized per chunk as [128, 8, sub] blocks (partition-major, row slots
5..12), so each pair store writes 2*sub*4 contiguous bytes per partition
(bigger DMA lines amortize per-packet overhead).  Host reassembles.

Sharding: x is split along N across the 8 cores; each core computes its 8
nonzero basis rows; host assembles the full [5, 13, 500000] output.
"""

import numpy as np

import concourse.bass as bass
import concourse.bacc as bacc
import concourse.tile as tile
from concourse import mybir
from concourse.bass_utils import run_bass_kernel_spmd
import concourse.dve_ops as dve_ops_mod
from concourse.dve_spec import (
    Spec, Src0, C0, C1, C2, Zero, One, relu, sq, maxx, lower, _has_src1,
)
from concourse.dve_uop import DveOpSpec

N_CORES = 8
N_ROWS = 5          # x rows
N_BASIS = 13        # output basis rows (rows 0..4 are zero)
R_LO = 5            # first nonzero basis row
N_NZ = N_BASIS - R_LO                # 8 nonzero rows
N_FULL = 500000
N_SHARD = N_FULL // N_CORES          # 62500
N_ELEM = N_ROWS * N_SHARD            # 312500 elements per core
P = 128                              # SBUF partitions (all 16 DMA engines)
FD = -(-N_ELEM // P)                 # 2442 elements per partition
N_PAD = P * FD                       # 312576
X_PAD_VAL = np.float32(10.0)         # maps to u far outside every support
C1V = float(np.float64(6.0) ** (-1.0 / 3.0))   # c with c^3 = 1/6
CHUNKS = (256, 1000, 1186)           # graded: small first chunk
# Per-chunk z assignment: row 6 always on DVE (z_op); row 7 on DVE for the
# first two chunks, ScalarE for the last (engine balance); rows 8-11 ScalarE.
V_ROWS_BY_CHUNK = ((6, 7), (6, 7), (6,))
SKIP_INIT_BARRIER = True
ENABLE_ASSERTS = True

assert sum(CHUNKS) == FD


def _chunks():
    bounds = [0]
    for c in CHUNKS:
        bounds.append(bounds[-1] + c)
    return list(zip(bounds[:-1], bounds[1:]))


def _register_dve_op(name, spec):
    for op in dve_ops_mod.OPS:
        if op.name == name:
            return op
    opcode = dve_ops_mod._CUSTOM_DVE_ROW_BASE + len(dve_ops_mod.OPS)
    assert opcode < 0x20, "custom DVE row overflow"
    shas = {}
    for ver in ("v3", "v4"):
        uops = lower(spec, ver=ver)
        shas[ver] = DveOpSpec(
            name=name, opcode=opcode, uops=uops, rd1_en=_has_src1(spec)
        ).sha(ver)
    op = dve_ops_mod.DveOp(name, spec, subdim=False, uops_sha=shas)
    dve_ops_mod.OPS.append(op)
    dve_ops_mod._SUB_OPCODE_FOR_NAME[name] = opcode
    dve_ops_mod.CUSTOM_DVE_SPECS[name] = spec
    return op


def _get_z_op():
    # out = relu((2 - |in0*imm2 + s0|) * s1)       (7 ALU stages)
    w = Src0 * C2 + C0
    a = maxx(w, Zero - w)
    body = relu(((One + One) - a) * C1)
    spec = Spec(
        body=body,
        reference=lambda in0, in1, s0, s1, imm2: np.maximum(
            (np.float32(2.0) - np.abs(in0 * imm2 + s0)) * s1, np.float32(0.0)
        ).astype(np.float32),
    )
    return _register_dve_op("BSPLINE_Z_ANT", spec)


def _get_cube_diff_op():
    # out = in0^3 - imm2 * relu(in0 - s0)^3        (8 ALU stages)
    r = relu(Src0 - C0)
    body = sq(Src0) * Src0 - sq(r) * r * C2
    spec = Spec(
        body=body,
        reference=lambda in0, in1, s0, s1, imm2: (
            in0.astype(np.float32) ** 3
            - np.maximum(in0 - s0, np.float32(0.0)).astype(np.float32) ** 3 * imm2
        ).astype(np.float32),
    )
    return _register_dve_op("BSPLINE_CUBE_DIFF_ANT", spec)


def _get_edge_cube_op():
    # out = relu(in0*s0 + s1)^3                    (5 ALU stages)
    r = relu(Src0 * C0 + C1)
    spec = Spec(
        body=sq(r) * r,
        reference=lambda in0, in1, s0, s1, imm2: (
            np.maximum(in0 * s0 + s1, np.float32(0.0)).astype(np.float32) ** 3
        ).astype(np.float32),
    )
    return _register_dve_op("BSPLINE_EDGE_CUBE_ANT", spec)


def _register_const(nc, value):
    """Make `value` usable as an activation bias (const_aps lookup).
    Must be called inside the TileContext: the memset is tracked by Tile."""
    f32 = mybir.dt.float32
    key = (f32, float(value))
    if key in nc.const_aps.aps:
        return
    t = nc.alloc_sbuf_tensor(f"const-f32-{float(value)}", [128, 1], f32)
    nc.vector.memset(t.ap(), float(value))
    nc.const_aps.aps[key] = t.ap()


def _build_bass():
    cube_diff_op = _get_cube_diff_op()
    edge_cube_op = _get_edge_cube_op()
    z_op = _get_z_op()
    f32 = mybir.dt.float32
    # Skip Bass.__init__'s trailing all-engine barrier (only guards its
    # 0.0/1.0 const memsets; the earlier _nrt_pseudo_barrier already orders
    # the semaphore clears).  The only in-kernel reader of those consts is
    # the throwaway table-warm activation below.  Saves ~2us of preamble.
    if SKIP_INIT_BARRIER:
        _orig_barrier = bass.Bass.all_engine_barrier
        bass.Bass.all_engine_barrier = lambda self: None
        try:
            nc = bacc.Bacc(
                "TRN2", target_bir_lowering=False, debug=False,
                num_devices=N_CORES, enable_asserts=ENABLE_ASSERTS,
            )
        finally:
            bass.Bass.all_engine_barrier = _orig_barrier
    else:
        nc = bacc.Bacc(
            "TRN2", target_bir_lowering=False, debug=False,
            num_devices=N_CORES, enable_asserts=ENABLE_ASSERTS,
        )
    x_dram = nc.dram_tensor("x", [N_PAD], f32, kind="ExternalInput")
    out_dram = nc.dram_tensor("out", [N_NZ * N_PAD], f32, kind="ExternalOutput")
    xv = x_dram.ap().rearrange("(p f) -> p f", p=P)

    with tile.TileContext(nc) as tc:
        with (
            tc.tile_pool(name="const", bufs=1) as cpool,
            tc.tile_pool(name="work", bufs=2) as wpool,
        ):
            x_tile = cpool.tile([P, FD], f32, tag="x")
            for (lo, hi) in _chunks():
                nc.sync.dma_start(out=x_tile[:, lo:hi], in_=xv[:, lo:hi])

            warm = cpool.tile([P, 1], f32, tag="warm")
            nc.scalar.activation(
                warm[:], nc.const_aps.aps[(f32, 0.0)][:P, :],
                mybir.ActivationFunctionType.Abs, bias=0.0, scale=1.0,
            )
            for r in range(7, 12):
                _register_const(nc, float(6 - r))
            _register_const(nc, 2.0 * C1V)

            for ci, (lo, hi) in enumerate(_chunks()):
                sub = hi - lo
                xs = x_tile[:, lo:hi]
                base = N_NZ * P * lo
                blk = out_dram.ap()[base: base + N_NZ * P * sub].rearrange(
                    "(p f) -> p f", p=P
                )  # [P, 8*sub], per-partition layout = row slots 5..12

                # --- DVE: edge rows straight from x ---
                o5 = wpool.tile([P, sub], f32, tag="o5", bufs=2)
                nc.vector._custom_dve(
                    edge_cube_op, out=o5[:], in0=xs,
                    s0=-5.0 * C1V, s1=C1V,
                )
                nc.sync.dma_start(out=blk[:, 0:sub], in_=o5[:])
                o12 = wpool.tile([P, sub], f32, tag="o12", bufs=2)
                nc.vector._custom_dve(
                    edge_cube_op, out=o12[:], in0=xs,
                    s0=5.0 * C1V, s1=-4.0 * C1V,
                )
                nc.sync.dma_start(out=blk[:, 7 * sub: 8 * sub], in_=o12[:])

                # --- z for rows 6..11 into stacked pair tiles; cube_diff per
                # pair; store per pair (2*sub contiguous bytes/partition) ---
                v_rows = V_ROWS_BY_CHUNK[ci]
                ztiles = {}
                for pi, pair in enumerate(((6, 7), (8, 9), (10, 11))):
                    zt = wpool.tile(
                        [P, 2 * sub], f32, tag=f"z{pair[0]}", bufs=2,
                        name=f"z{pair[0]}",
                    )
                    ztiles[pair] = zt
                    for off, r in zip((0, sub), pair):
                        if r in v_rows:
                            # z = relu((2 - |5x + (6-r)|) * c) in one DVE op
                            nc.vector._custom_dve(
                                z_op, out=zt[:, off:off + sub], in0=xs,
                                s0=float(6 - r), s1=C1V, imm2=5.0,
                            )
                        else:
                            a_t = wpool.tile(
                                [P, sub], f32, tag="a", bufs=3, name="a_t"
                            )
                            nc.scalar.activation(
                                a_t[:], xs, mybir.ActivationFunctionType.Abs,
                                bias=float(6 - r), scale=5.0,
                            )
                            nc.scalar.activation(
                                zt[:, off:off + sub], a_t[:],
                                mybir.ActivationFunctionType.Relu,
                                bias=2.0 * C1V, scale=-C1V,
                            )
                    o_t = wpool.tile(
                        [P, 2 * sub], f32, tag=f"o{pair[0]}", bufs=2,
                        name=f"o{pair[0]}",
                    )
                    nc.vector._custom_dve(
                        cube_diff_op, out=o_t[:], in0=zt[:],
                        s0=C1V, imm2=4.0,
                    )
                    s0_off = (1 + 2 * pi) * sub
                    nc.sync.dma_start(
                        out=blk[:, s0_off: s0_off + 2 * sub], in_=o_t[:]
                    )
    nc.compile()
    return nc


_NC_CACHE = None


def _get_nc():
    global _NC_CACHE
    if _NC_CACHE is None:
        _NC_CACHE = _build_bass()
    return _NC_CACHE


def kernel(x, grid=None, k=None, **_ignored):
    x = np.asarray(x, dtype=np.float32)
    assert x.shape == (N_ROWS, N_FULL), x.shape
    nc = _get_nc()
    in_maps = []
    for i in range(N_CORES):
        sh = np.full(N_PAD, X_PAD_VAL, dtype=np.float32)
        sh[:N_ELEM] = np.ascontiguousarray(
            x[:, i * N_SHARD: (i + 1) * N_SHARD]
        ).reshape(-1)
        in_maps.append({"x": sh})
    res = run_bass_kernel_spmd(nc, in_maps, list(range(N_CORES))).results
    full = np.zeros((N_ROWS, N_BASIS, N_FULL), dtype=np.float32)
    for i in range(N_CORES):
        o = np.asarray(res[i]["out"]).reshape(-1)  # [8 * N_PAD] chunk blocks
        r8 = np.empty((N_NZ, P, FD), dtype=np.float32)
        for (lo, hi) in _chunks():
            sub = hi - lo
            blk = o[N_NZ * P * lo: N_NZ * P * hi].reshape(P, N_NZ, sub)
            r8[:, :, lo:hi] = blk.transpose(1, 0, 2)
        rows = r8.reshape(N_NZ, N_PAD)[:, :N_ELEM]
        full[:, R_LO:, i * N_SHARD: (i + 1) * N_SHARD] = rows.reshape(
            N_NZ, N_ROWS, N_SHARD
        ).transpose(1, 0, 2)
    return full
